# revision 33
# baseline (speedup 1.0000x reference)
"""Trainium2 Bass kernel for FANS neural-output update (banded per-subnet MLPs).

Math (per batch row x in R^32, 32 subnets):
    z_n   = x[idx[n]]                 # 8 selected states (circulant band)
    h_n   = tanh(z_n @ W1[n] + b1[n]) # [32] hidden
    y[n]  = h_n @ W2[n] + b2[n]       # scalar per subnet

Implementation: data-parallel over 8 NeuronCores (batch-sharded). PE matmuls
run in float32r (TF32-like, full rate at N>=256 vs 1/4 rate for fp32).
Per core, per 2048-row megatile:
  - DMA x in as S[128, 512] (16 consecutive rows per partition, 2KB contig)
  - DVE 32x32 block-transpose -> X^T tiles [32 states, 512 batch]
    (batch order within columns is a fixed permutation, mirrored on output)
  - DVE splits x into f32r hi + residual lo; SBUF->SBUF DMAs assemble
    t96 = [x_hi; x_hi; x_lo] per group (K-stacked residual compensation)
  - PE layer 1: K=96 against [W1hi; W1lo; W1hi] (banded W1 pre-expanded to
    dense [32, 1024] on host) -> PSUM; K-stacking cancels both x and W1
    f32r rounding at zero extra streaming cost (matmul cost ~ N only)
  - ACT: tanh fused PSUM->SBUF, f32r out (the wall: 134M tanh, 128/cyc/core)
  - PE layer 2: block-diag W2 chunks [128, 32], K=128 accumulation into
    per-group Y^T [32, 512] PSUM tiles (f32r mms require dst partition 0)
  - DVE copy PSUM->SBUF, DVE block-transpose back, DMA out (2KB contig)
"""

import numpy as np
from contextlib import ExitStack

import concourse.bass as bass
import concourse.tile as tile
from concourse import bacc, mybir
from concourse.bass_utils import run_bass_kernel_spmd

N_CORES = 8
NXB = 32   # state dim
NXO = 32   # subnets / outputs
NF = 32    # hidden width per subnet
P = 128
TT = 16                 # rows per partition in a megatile
MEGA = P * TT           # 2048 rows per megatile
NB = 512                # batch columns per group tile
NCHUNK = 8              # f-chunks of 128 (= 1024/128)
CSETS = ((0, 1), (2, 3), (4, 5), (6, 7))

F32 = mybir.dt.float32
F32R = mybir.dt.float32r

_module_cache = {}


def _round_f32r(a: np.ndarray) -> np.ndarray:
    """Round fp32 to the FP32R grid (low 13 mantissa bits cleared, RNE-ish)."""
    u = np.ascontiguousarray(a, np.float32).view(np.uint32)
    u = (u + np.uint32(0x0FFF) + ((u >> np.uint32(13)) & np.uint32(1))) & np.uint32(
        0xFFFFE000
    )
    return u.view(np.float32)


def _build_module(bpc: int, has_b1: bool, has_b2: bool):
    """Build + bacc-compile the per-core Bass module for bpc rows/core."""
    assert bpc % MEGA == 0
    nmega = bpc // MEGA

    nc = bacc.Bacc(
        "TRN2",
        target_bir_lowering=False,
        debug=False,
        enable_asserts=False,
        num_devices=N_CORES,
    )
    x = nc.dram_tensor("x", [bpc, NXB], F32, kind="ExternalInput").ap()
    w1 = nc.dram_tensor("w1", [96, NXO * NF], F32R, kind="ExternalInput").ap()
    w2 = nc.dram_tensor("w2", [P, NCHUNK * NXO], F32R, kind="ExternalInput").ap()
    b1d = b2d = None
    if has_b1:
        b1d = nc.dram_tensor("b1", [1, NCHUNK * P], F32, kind="ExternalInput").ap()
    if has_b2:
        b2d = nc.dram_tensor("b2", [P, 1], F32, kind="ExternalInput").ap()
    y = nc.dram_tensor("y", [bpc, NXO], F32, kind="ExternalOutput").ap()

    xv = x.rearrange("(m p t) k -> m p (t k)", p=P, t=TT)
    yv = y.rearrange("(m i a j) k -> m (i a) (j k)", i=4, a=NXB, j=TT)

    with tile.TileContext(nc) as tc, ExitStack() as ctx:
        singles = ctx.enter_context(tc.tile_pool(name="singles", bufs=1))
        xin = ctx.enter_context(tc.tile_pool(name="xin", bufs=3))
        xt = ctx.enter_context(tc.tile_pool(name="xt", bufs=3))
        xth = ctx.enter_context(tc.tile_pool(name="xth", bufs=3))
        xtl = ctx.enter_context(tc.tile_pool(name="xtl", bufs=3))
        xt96 = ctx.enter_context(tc.tile_pool(name="xt96", bufs=3))
        hps = ctx.enter_context(tc.tile_pool(name="hps", bufs=3, space="PSUM"))
        yps = ctx.enter_context(tc.tile_pool(name="yps", bufs=2, space="PSUM"))
        hsb = ctx.enter_context(tc.tile_pool(name="hsb", bufs=6))
        ysb = ctx.enter_context(tc.tile_pool(name="ysb", bufs=3))
        usb = ctx.enter_context(tc.tile_pool(name="usb", bufs=3))

        w1sb = singles.tile([96, NXO * NF], F32R)
        nc.gpsimd.dma_start(w1sb[:], w1[:])
        w1lo0 = singles.tile([32, NXO * NF], F32R)
        nc.gpsimd.dma_start(w1lo0[:], w1[32:64, :])
        w2sb = singles.tile([P, NCHUNK * NXO], F32R)
        nc.gpsimd.dma_start(w2sb[:], w2[:])
        if has_b1:
            b1sb = singles.tile([1, NCHUNK * P], F32)
            nc.sync.dma_start(b1sb[:], b1d[:])
            ones = singles.tile([1, NB], F32)
            nc.vector.memset(ones[:], 1.0)
        if has_b2:
            b2sb = singles.tile([P, 1], F32)
            nc.sync.dma_start(b2sb[:], b2d[:])

        for m in range(nmega):
            # Entire input chain is split per 32-row group so group 0's data
            # reaches the PE quickly (shrinks the pipeline head stall on ACT).
            s = xin.tile([P, TT * NXB], F32)
            t0 = xt.tile([P, TT * NXB], F32)
            th = xth.tile([P, TT * NXB], F32R)
            tl = xtl.tile([P, TT * NXB], F32R)
            # t96 rows [x_hi; x_hi; x_lo], free dim = (group g, 512 batch cols)
            t96 = xt96.tile([96, 4 * NB], F32R)
            # group 0 fast path first (shrinks the pipeline-head stall on ACT),
            # then groups 1-3 in one wide pass each (DVE cost ~ free size only).
            g0, rest = slice(0, 32), slice(32, P)
            nc.sync.dma_start(s[:], xv[m])
            nc.vector.transpose(t0[g0, :], s[g0, :])
            nc.vector.tensor_copy(th[g0, :], t0[g0, :])
            nc.vector.tensor_sub(tl[g0, :], t0[g0, :], th[g0, :])
            nc.sync.dma_start(t96[0:32, 0:NB], th[g0, :])
            nc.sync.dma_start(t96[32:64, 0:NB], th[g0, :])
            nc.sync.dma_start(t96[64:96, 0:NB], tl[g0, :])
            for g in range(1, 4):
                nc.vector.transpose(t0[32 * g:32 * g + 32, :],
                                    s[32 * g:32 * g + 32, :])
            for g in range(1, 4):
                sl = slice(32 * g, 32 * g + 32)
                nc.vector.tensor_copy(th[sl, :], t0[sl, :])
                nc.vector.tensor_sub(tl[sl, :], t0[sl, :], th[sl, :])
            for g in range(1, 4):
                sl = slice(32 * g, 32 * g + 32)
                fr = slice(NB * g, NB * (g + 1))
                nc.sync.dma_start(t96[0:32, fr], th[sl, :])
                nc.sync.dma_start(t96[32:64, fr], th[sl, :])
                nc.sync.dma_start(t96[64:96, fr], tl[sl, :])

            yc = ysb.tile([P, NB], F32)
            for gpair in ((0, 1), (2, 3)):
                ypts = {
                    g: yps.tile([NXO, NB], F32, name=f"ypt{m}_{g}", tag="ypt")
                    for g in gpair
                }
                for ci, cs in enumerate(CSETS):
                    for g in gpair:
                        # Head fast path: the kernel's very first chunk group
                        # reads th/tl directly as three K=32 accumulating
                        # matmuls, skipping the wait on the t96 remap DMAs.
                        fast = m == 0 and g == 0 and ci == 0
                        hp = hps.tile([P, NB * len(cs)], F32)
                        for j, c in enumerate(cs):
                            ck = slice(128 * c, 128 * (c + 1))
                            if fast:
                                nc.tensor.matmul(
                                    hp[:, j * NB:(j + 1) * NB],
                                    w1sb[0:32, ck], th[0:32, :],
                                    start=True, stop=False,
                                )
                                nc.tensor.matmul(
                                    hp[:, j * NB:(j + 1) * NB],
                                    w1lo0[:, ck], th[0:32, :],
                                    start=False, stop=False,
                                )
                                nc.tensor.matmul(
                                    hp[:, j * NB:(j + 1) * NB],
                                    w1sb[0:32, ck], tl[0:32, :],
                                    start=False, stop=not has_b1,
                                )
                                continue
                            nc.tensor.matmul(
                                hp[:, j * NB:(j + 1) * NB],
                                w1sb[:, 128 * c:128 * (c + 1)],
                                t96[:, NB * g:NB * (g + 1)],
                                start=True,
                                stop=not has_b1,
                            )
                            if has_b1:
                                nc.tensor.matmul(
                                    hp[:, j * NB:(j + 1) * NB],
                                    b1sb[0:1, 128 * c:128 * (c + 1)],
                                    ones[0:1, :],
                                    start=False,
                                    stop=True,
                                )
                        hs = hsb.tile([P, NB * len(cs)], F32R)
                        nc.scalar.activation(
                            hs[:], hp[:], mybir.ActivationFunctionType.Tanh
                        )
                        for j, c in enumerate(cs):
                            nc.tensor.matmul(
                                ypts[g][:, :],
                                w2sb[:, 32 * c:32 * (c + 1)],
                                hs[:, j * NB:(j + 1) * NB],
                                start=(c == 0),
                                stop=(c == NCHUNK - 1),
                            )
                for g in gpair:
                    if has_b2:
                        nc.vector.tensor_scalar_add(
                            yc[32 * g:32 * g + 32, :], ypts[g][:], b2sb[0:NXO, 0:1]
                        )
                    else:
                        nc.vector.tensor_copy(yc[32 * g:32 * g + 32, :], ypts[g][:])
                # drain this half as soon as it is complete (shorter tail)
                hsl = slice(32 * gpair[0], 32 * gpair[1] + 32)
                u = usb.tile([P, NB], F32, name=f"u{m}_{gpair[0]}", tag="u")
                nc.vector.transpose(u[hsl, :], yc[hsl, :])
                nc.sync.dma_start(yv[m][hsl, :], u[hsl, :])

    nc.compile()
    return nc


def _prep_weights(W1, b1, W2, b2, idx):
    W1 = np.asarray(W1, np.float32)
    W2 = np.asarray(W2, np.float32)
    b1 = np.asarray(b1, np.float32).reshape(NXO, NF)
    b2 = np.asarray(b2, np.float32).reshape(NXO)
    idx = np.asarray(idx)

    # Banded -> dense first-layer weight, columns ordered (n, f).
    w1cm = np.zeros((NXB, NXO * NF), np.float32)
    for n in range(NXO):
        for k in range(idx.shape[1]):
            w1cm[idx[n, k], 32 * n:32 * n + 32] += W1[n, k, :]
    # K=96 residual-compensated stack: rows [W1hi; W1lo; W1hi] pairs with
    # rhs rows [x_hi; x_hi; x_lo] so both x and W1 f32r-rounding cancel.
    w1hi = _round_f32r(w1cm)
    w1lo = _round_f32r(w1cm - w1hi)
    w1cat = np.concatenate([w1hi, w1lo, w1hi], axis=0)  # [96, 1024]

    # Block-diagonal second-layer weight, one [128, 32] block per f-chunk.
    w2flat = np.zeros((P, NCHUNK * NXO), np.float32)
    for c in range(NCHUNK):
        for r in range(4):
            n = 4 * c + r
            w2flat[32 * r:32 * r + 32, 32 * c + n] = W2[n, :, 0]

    has_b1 = bool(np.any(b1 != 0.0))
    has_b2 = bool(np.any(b2 != 0.0))
    b1flat = None
    if has_b1:
        # b1flat[0, 128c + p] = b1[4c + p//32, p%32]
        b1flat = np.zeros((1, NCHUNK * P), np.float32)
        for c in range(NCHUNK):
            for r in range(4):
                b1flat[0, 128 * c + 32 * r:128 * c + 32 * r + 32] = b1[4 * c + r]
    b2vec = None
    if has_b2:
        b2vec = np.tile(b2.reshape(1, NXO), (4, 1)).reshape(P, 1).astype(np.float32)

    return w1cat, w2flat, b1flat, b2vec, has_b1, has_b2


def _get_module(bpc, has_b1, has_b2):
    key = (bpc, has_b1, has_b2)
    if key not in _module_cache:
        _module_cache[key] = _build_module(bpc, has_b1, has_b2)
    return _module_cache[key]


def make_in_maps(x_b, W1, b1, W2, b2, idx):
    x_b = np.ascontiguousarray(np.asarray(x_b, np.float32))
    B = x_b.shape[0]
    assert B % N_CORES == 0
    bpc = B // N_CORES
    w1cat, w2flat, b1flat, b2vec, has_b1, has_b2 = _prep_weights(W1, b1, W2, b2, idx)
    in_maps = []
    for i in range(N_CORES):
        m = {
            "x": np.ascontiguousarray(x_b[i * bpc:(i + 1) * bpc]),
            "w1": w1cat,
            "w2": _round_f32r(w2flat),
        }
        if has_b1:
            m["b1"] = b1flat
        if has_b2:
            m["b2"] = b2vec
        in_maps.append(m)
    return in_maps, bpc, has_b1, has_b2


def kernel(x_b, W1, b1, W2, b2, idx):
    in_maps, bpc, has_b1, has_b2 = make_in_maps(x_b, W1, b1, W2, b2, idx)
    nc = _get_module(bpc, has_b1, has_b2)
    res = run_bass_kernel_spmd(nc, in_maps, core_ids=list(range(N_CORES)))
    out = np.concatenate([res.results[i]["y"] for i in range(N_CORES)], axis=0)
    return out.astype(np.float32)
